# revision 3
# baseline (speedup 1.0000x reference)
"""Causal self-attention with RoPE (B=2, T=2048, C=1024, H=16, D=64) on 8
Trainium2 NeuronCores.

Sharding: tensor-parallel over heads - each core owns 2 heads (QKV and output
projections sliced on the head axis); the per-core partial outputs (full
[C, B*T] each, fp16) are summed on the host in fp32.

v2 changes vs the fp32r baseline:
  - fp16 datapath end to end (xt/wqkv/wout/q/k/v/p/y/outT) halves DMA traffic
    (40 MB -> ~21 MB per core), halves LDWEIGHTS, removes the fp32r small-N
    4x matmul penalty, and enables the DVE 2x 16-bit mode for RoPE add/subs.
  - QKV chunk i and attention chunk i are interleaved so the PE instruction
    stream has no phase gaps (keeps the tensor engine p-state high).
  - scores for both heads land in one 2-bank PSUM tile [128, 2, 512] so a
    single ACT exp covers both heads per kj chunk (halves ACT instruction
    count on the softmax stream).

Per-core layout (everything transposed: features on partitions, tokens free):
  xT [1024, 4096] fp16 -> QKV chunks via PE matmul, W rows pre-permuted into
  E/O/V 128-row groups; RoPE on DVE; scores S^T per (kj, head) into the
  shared PSUM tile; exp on ACT (scale=1/8), causal diag via gpsimd
  affine_select; PV accumulates y^T plus denominator rows via 64 ones
  columns in v; out proj partial [1024, 4096] fp16 DMA'd out.
"""

import sys
import types

import numpy as np

import concourse.bass as bass
import concourse.tile as tile
from concourse import bacc
from concourse import mybir
from concourse.bass_utils import run_bass_kernel_spmd
from concourse.masks import make_identity

F32 = mybir.dt.float32
F16 = mybir.dt.float16

B = 2
T = 2048
C = 1024
D = 64
N_CORES = 8
BT = B * T              # 4096
TC = 512                # token chunk (free dim of most matmuls)
NQI = T // TC           # 4 qi chunks per batch
NKJ = T // 128          # 16 kj chunks per batch
KC = C // 128           # 8 contraction chunks for the projections


def _install_ntff_hook():
    """bass_utils imports antenv.axon_hooks when tracing; this image lacks it.
    Recreate it from the ctypes NTFF driver so trace=True works."""
    if "antenv.axon_hooks" in sys.modules:
        return
    try:
        from trn_agent_boot.trn_boot import _ntff_profile_via_ctypes

        hook = _ntff_profile_via_ctypes("/opt/axon/libaxon_pjrt.so")
    except Exception:
        hook = None
    mod = types.ModuleType("antenv.axon_hooks")
    mod.get_axon_ntff_profile_hook = lambda: hook
    mod.set_axon_ntff_profile_hook = lambda h: None
    sys.modules["antenv.axon_hooks"] = mod


_install_ntff_hook()

X_BUFS = 3
PIPE_DEPTH = 1


def build_nc():
    nc = bacc.Bacc(None, target_bir_lowering=False, debug=False)

    xt = nc.declare_dram_parameter("xt", [128, (BT // TC) * KC * TC], F16, isOutput=False)
    wqkv = nc.declare_dram_parameter("wqkv", [128, KC * 384], F16, isOutput=False)
    wout = nc.declare_dram_parameter("wout", [128, C], F16, isOutput=False)
    cs = nc.declare_dram_parameter("cs", [128, 2 * T], F32, isOutput=False)
    ones = nc.declare_dram_parameter("ones", [128, 64], F16, isOutput=False)
    outT = nc.declare_dram_parameter("outT", [C, BT], F16, isOutput=True)

    with tile.TileContext(nc) as tc:
        with (
            tc.sbuf_pool(name="statics", bufs=1) as statics,
            tc.sbuf_pool(name="pool_x", bufs=X_BUFS) as pool_x,
            tc.sbuf_pool(name="pool_rope", bufs=2) as pool_rope,
            tc.sbuf_pool(name="pool_qk", bufs=2) as pool_qk,
            tc.sbuf_pool(name="pool_v", bufs=2) as pool_v,
            tc.sbuf_pool(name="pool_y", bufs=2) as pool_y,
            tc.sbuf_pool(name="pool_vs", bufs=2) as pool_vs,
            tc.sbuf_pool(name="pool_p", bufs=6) as pool_p,
            tc.sbuf_pool(name="pool_o", bufs=3) as pool_o,
            tc.sbuf_pool(name="pool_rb", bufs=2) as pool_rb,
            tc.psum_pool(name="ps_mm", bufs=2) as ps_mm,
            tc.psum_pool(name="ps_st", bufs=2) as ps_st,
            tc.psum_pool(name="ps_y", bufs=2) as ps_y,
        ):
            ident = statics.tile([128, 128], F16)
            make_identity(nc, ident)

            wqkv_sb = statics.tile([128, KC * 384], F16)
            nc.sync.dma_start(out=wqkv_sb, in_=wqkv[:, :])
            # deferred statics: emitted after the first xt chunk's DMA so the
            # first QKV matmuls aren't queued behind the table loads
            wout_sb = statics.tile([128, C], F16)
            cos_sb = statics.tile([128, T], F32)
            sin_sb = statics.tile([128, T], F32)
            statics_emitted = []

            def emit_deferred_statics():
                if statics_emitted:
                    return
                statics_emitted.append(1)
                nc.sync.dma_start(out=cos_sb, in_=cs[:, 0:T])
                nc.sync.dma_start(out=sin_sb, in_=cs[:, T : 2 * T])
                nc.sync.dma_start(out=wout_sb, in_=wout[:, :])

            def qkv_chunk(b, tci, q_t, k_t, v_all):
                g = 4 * b + tci
                xt_sb = pool_x.tile([128, KC, TC], F16, tag="x", name=f"xt_{g}")
                nc.sync.dma_start(
                    out=xt_sb,
                    in_=xt[:, g * KC * TC : (g + 1) * KC * TC].rearrange(
                        "p (kc n) -> p kc n", n=TC
                    ),
                )
                emit_deferred_statics()
                if tci == 0:
                    # ones columns of v_all (64 per head per 256-block)
                    vm = v_all.rearrange("p (m c) -> p m c", c=128)
                    ones_bc = bass.AP(
                        tensor=ones, offset=0, ap=[[64, 128], [0, 2 * NKJ], [1, 64]]
                    )
                    nc.sync.dma_start(out=vm[:, :, 64:128], in_=ones_bc)
                # V first: its PSUM slot is freed by the copy below while
                # E/O are still being produced (lets ps_mm run with bufs=2)
                psv = ps_mm.tile([128, TC], F32, tag="mm", name=f"psv_{g}")
                pse = ps_mm.tile([128, TC], F32, tag="mm", name=f"pse_{g}")
                pso = ps_mm.tile([128, TC], F32, tag="mm", name=f"pso_{g}")
                for mi, ps in ((2, psv), (0, pse), (1, pso)):
                    for kc in range(KC):
                        nc.tensor.matmul(
                            ps,
                            wqkv_sb[
                                :, kc * 384 + 128 * mi : kc * 384 + 128 * (mi + 1)
                            ],
                            xt_sb[:, kc, :],
                            start=(kc == 0),
                            stop=(kc == KC - 1),
                        )
                # v: PSUM -> SBUF fp16, then transpose 128x128 blocks into v_all
                v_sb = pool_vs.tile([128, TC], F16, tag="vs", name=f"vsb_{g}")
                nc.scalar.activation(
                    out=v_sb, in_=psv, func=mybir.ActivationFunctionType.Copy
                )
                va = v_all.rearrange("p (n h c) -> p n h c", h=2, c=128)
                for s in range(4):
                    j = 4 * tci + s
                    tr = ps_y.tile([128, 128], F16, tag="y", name=f"tr_{g}_{s}")
                    nc.tensor.transpose(
                        tr, v_sb[:, 128 * s : 128 * (s + 1)], ident
                    )
                    nc.scalar.activation(
                        out=va[:, j, :, 0:64],
                        in_=tr.rearrange("p (h c) -> p h c", h=2),
                        func=mybir.ActivationFunctionType.Copy,
                    )

                # RoPE
                c_sl = cos_sb[:, tci * TC : (tci + 1) * TC]
                s_sl = sin_sb[:, tci * TC : (tci + 1) * TC]
                tEC = pool_rope.tile([128, TC], F16, tag="tEC", name=f"tEC_{g}")
                nc.vector.tensor_mul(out=tEC, in0=pse, in1=c_sl)
                tES = pool_rope.tile([128, TC], F16, tag="tES", name=f"tES_{g}")
                nc.vector.tensor_mul(out=tES, in0=pse, in1=s_sl)
                tOS = pool_rope.tile([128, TC], F16, tag="tOS", name=f"tOS_{g}")
                nc.vector.tensor_mul(out=tOS, in0=pso, in1=s_sl)
                tOC = pool_rope.tile([128, TC], F16, tag="tOC", name=f"tOC_{g}")
                nc.vector.tensor_mul(out=tOC, in0=pso, in1=c_sl)

                sl = slice(tci * TC, (tci + 1) * TC)
                # q_t rows [h0e|h0o|h1e|h1o]; E rows [q0e|q1e|k0e|k1e]
                for h in range(2):
                    he = slice(32 * h, 32 * (h + 1))
                    nc.vector.tensor_sub(
                        out=q_t[64 * h : 64 * h + 32, sl],
                        in0=tEC[he], in1=tOS[he],
                    )
                    nc.vector.tensor_add(
                        out=q_t[64 * h + 32 : 64 * h + 64, sl],
                        in0=tES[he], in1=tOC[he],
                    )
                    ke_ = slice(64 + 32 * h, 64 + 32 * (h + 1))
                    nc.vector.tensor_sub(
                        out=k_t[64 * h : 64 * h + 32, sl],
                        in0=tEC[ke_], in1=tOS[ke_],
                    )
                    nc.vector.tensor_add(
                        out=k_t[64 * h + 32 : 64 * h + 64, sl],
                        in0=tES[ke_], in1=tOC[ke_],
                    )

            def attention_chunk(b, i, q_t, k_t, v_all, y_t):
                nj = 4 * i + 4
                yaccs = {}
                for h in range(2):
                    yaccs[h] = ps_y.tile(
                        [128, TC], F32, tag="y", name=f"yacc_{b}_{i}_{h}"
                    )

                def st_of(j):
                    r = j - 4 * i
                    return 128 * r if r > 0 else 0

                # software-pipelined by one step: PE issues S(j),S(j),
                # PV(j-1),PV(j-1) back-to-back while exp(j) runs on ACT
                p_tiles = {}
                for j in range(nj + PIPE_DEPTH):
                    if j < nj:
                        st = st_of(j)
                        r = j - 4 * i
                        ksl = slice(128 * j, 128 * (j + 1))
                        qsl = slice(TC * i + st, TC * (i + 1))
                        ps_s = ps_st.tile(
                            [128, 2, TC], F32, tag="st", name=f"s_{b}_{i}_{j}"
                        )
                        for h in range(2):
                            hs = slice(64 * h, 64 * (h + 1))
                            nc.tensor.matmul(
                                ps_s[:, h, st:], k_t[hs, ksl], q_t[hs, qsl],
                                start=True, stop=True,
                            )
                        p_sb = pool_p.tile(
                            [128, 2, TC], F16, tag="p", name=f"p_{b}_{i}_{j}"
                        )
                        p_tiles[j] = p_sb
                        # one exp for both heads (2-bank PSUM source)
                        nc.scalar.activation(
                            out=p_sb[:, :, st:], in_=ps_s[:, :, st:],
                            func=mybir.ActivationFunctionType.Exp,
                            scale=0.125,
                        )
                        if r >= 0:
                            for h in range(2):
                                nc.gpsimd.affine_select(
                                    out=p_sb[:, h, st : st + 128],
                                    in_=p_sb[:, h, st : st + 128],
                                    pattern=[[1, 128]],
                                    channel_multiplier=-1,
                                    base=0,
                                    compare_op=mybir.AluOpType.is_ge,
                                    fill=0.0,
                                )
                    if j >= PIPE_DEPTH:
                        jp = j - PIPE_DEPTH
                        st = st_of(jp)
                        pp = p_tiles.pop(jp)
                        for h in range(2):
                            nc.tensor.matmul(
                                yaccs[h][:, st:],
                                v_all[
                                    :, 256 * jp + 128 * h : 256 * jp + 128 * (h + 1)
                                ],
                                pp[:, h, st:],
                                start=(jp == 0),
                                stop=(jp == nj - 1),
                            )
                for h in range(2):
                    rb = pool_rb.tile(
                        [128, TC], F32, tag="rb", name=f"rb_{b}_{i}_{h}"
                    )
                    # full-tile: the custom op mislowers nonzero base
                    # partitions; rows 0:64 are unused garbage recips
                    nc.vector.reciprocal_approx_fast(out=rb, in_=yaccs[h])
                    nc.vector.tensor_mul(
                        out=y_t[64 * h : 64 * (h + 1), TC * i : TC * (i + 1)],
                        in0=yaccs[h][0:64, :],
                        in1=rb[64:128],
                    )
                outproj_chunk(b, i, y_t)

            def outproj_chunk(b, tci, y_t):
                g = 4 * b + tci
                for cc in range(KC):
                    ps = ps_mm.tile([128, TC], F32, tag="mm", name=f"op_{g}_{cc}")
                    nc.tensor.matmul(
                        ps,
                        wout_sb[:, 128 * cc : 128 * (cc + 1)],
                        y_t[:, TC * tci : TC * (tci + 1)],
                        start=True,
                        stop=True,
                    )
                    o_sb = pool_o.tile([128, TC], F16, tag="o", name=f"o_{g}_{cc}")
                    if cc % 2 == 0:
                        nc.scalar.activation(
                            out=o_sb, in_=ps,
                            func=mybir.ActivationFunctionType.Copy,
                        )
                    else:
                        nc.vector.tensor_copy(out=o_sb, in_=ps)
                    nc.sync.dma_start(
                        out=outT[
                            128 * cc : 128 * (cc + 1), g * TC : (g + 1) * TC
                        ],
                        in_=o_sb,
                    )

            for b in range(B):
                q_t = pool_qk.tile([128, T], F16, tag="q", name=f"q_{b}")
                k_t = pool_qk.tile([128, T], F16, tag="k", name=f"k_{b}")
                v_all = pool_v.tile([128, 256 * NKJ], F16, tag="v", name=f"v_{b}")
                y_t = pool_y.tile([128, T], F16, tag="yt", name=f"y_{b}")

                for i in range(NQI):
                    qkv_chunk(b, i, q_t, k_t, v_all)
                    attention_chunk(b, i, q_t, k_t, v_all, y_t)

    nc.compile()
    return nc


_NC_CACHE = None


def _get_nc():
    global _NC_CACHE
    if _NC_CACHE is None:
        _NC_CACHE = build_nc()
    return _NC_CACHE


def _host_prep(x, qkv_w, out_w):
    x = np.asarray(x, dtype=np.float32)
    qkv_w = np.asarray(qkv_w, dtype=np.float32)
    out_w = np.asarray(out_w, dtype=np.float32)

    # xt[p, ((g*KC)+kc)*TC + n] = x[g*TC + n, kc*128 + p] - one contiguous
    # line per (partition, chunk) for the per-chunk DMA
    xt = np.ascontiguousarray(
        x.reshape(BT // TC, TC, KC, 128).transpose(3, 0, 2, 1).reshape(128, -1)
    ).astype(np.float16)

    # rope tables: row p uses frequency index p % 32
    t_idx = np.arange(T, dtype=np.float64)
    inv_freq = 1.0 / (10000.0 ** (np.arange(0, D, 2, dtype=np.float64) / D))  # 32
    ang = np.outer(np.tile(inv_freq, 4), t_idx)  # [128, T]
    cs = np.concatenate(
        [np.cos(ang), np.sin(ang)], axis=1
    ).astype(np.float32)  # [128, 2T]

    ones = np.ones((128, 64), np.float16)

    in_maps = []
    for core in range(N_CORES):
        h0 = 2 * core
        h1 = h0 + 1
        ev = np.arange(0, D, 2)
        od = np.arange(1, D, 2)
        e_rows = np.concatenate(
            [h0 * D + ev, h1 * D + ev, C + h0 * D + ev, C + h1 * D + ev]
        )
        o_rows = np.concatenate(
            [h0 * D + od, h1 * D + od, C + h0 * D + od, C + h1 * D + od]
        )
        v_rows = np.concatenate(
            [2 * C + h0 * D + np.arange(D), 2 * C + h1 * D + np.arange(D)]
        )
        rows = np.concatenate([e_rows, o_rows, v_rows])  # [384]
        w_part = qkv_w[rows]  # [384, C]
        # wqkv[p, kc*384 + m] = w_part[m, kc*128 + p]
        wqkv_c = np.ascontiguousarray(
            w_part.T.reshape(KC, 128, 384).transpose(1, 0, 2).reshape(128, KC * 384)
        ).astype(np.float16)
        cols = np.concatenate([h0 * D + np.arange(D), h1 * D + np.arange(D)])
        wout_c = np.ascontiguousarray(out_w[:, cols].T).astype(np.float16)  # [128, C]
        in_maps.append(
            {"xt": xt, "wqkv": wqkv_c, "wout": wout_c, "cs": cs, "ones": ones}
        )
    return in_maps


def _run(in_maps, trace=False):
    nc = _get_nc()
    return run_bass_kernel_spmd(
        nc, in_maps, core_ids=list(range(N_CORES)), trace=trace
    )


def kernel(x, qkv_w, out_w, _trace=False, _results_box=None):
    in_maps = _host_prep(x, qkv_w, out_w)
    res = _run(in_maps, trace=_trace)
    if _results_box is not None:
        _results_box.append(res)
    acc = np.zeros((C, BT), np.float32)
    for r in res.results:
        acc += r["outT"].astype(np.float32)
    out = acc.T.reshape(B, T, C)
    return np.ascontiguousarray(out)


# revision 6
# speedup vs baseline: 1.2639x; 1.2639x over previous
"""Causal self-attention with RoPE (B=2, T=2048, C=1024, H=16, D=64) on 8
Trainium2 NeuronCores.

Sharding: tensor-parallel over heads - each core owns 2 heads (QKV and output
projections sliced on the head axis); the per-core partial outputs (full
[C, B*T] each, fp16) are summed on the host in fp32.

v3 changes vs the fp32r baseline:
  - fp16 datapath end to end (halves DMA, LDWEIGHTS, removes fp32r small-N
    penalty, enables DVE 2x 16-bit mode for the RoPE add/subs).
  - causal masking is a PE matmul accumulate (identity @ (-240*tril) into the
    diagonal score blocks before exp) instead of a gpsimd affine_select after
    exp - the softmax chain is now S -> exp -> PV with no gpsimd hop.
  - QKV chunk i+1 is emitted piecewise INTO attention chunk i's step loop so
    the tensor engine always has independent matmuls to chew on while ACT
    works through exp (keeps the PE p-state high); the exp for both heads is
    one ACT instruction on a 2-bank PSUM tile [128, 2, 512].
  - out-proj PSUM->SBUF fp16 copies split between gpsimd and DVE; ACT keeps
    only exp + v copies.

Per-core layout (everything transposed: features on partitions, tokens free):
  xT [1024, 4096] fp16 -> QKV chunks via PE matmul, W rows pre-permuted into
  E/O/V 128-row groups; RoPE on DVE; scores S^T per (kj, head); PV
  accumulates y^T plus denominator rows via 64 ones columns in v; out proj
  partial [1024, 4096] fp16 DMA'd out.
"""

import sys
import types

import numpy as np

import concourse.bass as bass
import concourse.tile as tile
from concourse import bacc
from concourse import mybir
from concourse.bass_utils import run_bass_kernel_spmd
from concourse.masks import make_identity

F32 = mybir.dt.float32
F16 = mybir.dt.float16

B = 2
T = 2048
C = 1024
D = 64
N_CORES = 8
BT = B * T              # 4096
TC = 512                # token chunk (free dim of most matmuls)
NQI = T // TC           # 4 qi chunks per batch
NKJ = T // 128          # 16 kj chunks per batch
KC = C // 128           # 8 contraction chunks for the projections
MASK_VAL = -240.0       # exp(0.125*(s+MASK_VAL)) underflows fp16 to 0


def _install_ntff_hook():
    """bass_utils imports antenv.axon_hooks when tracing; this image lacks it.
    Recreate it from the ctypes NTFF driver so trace=True works."""
    if "antenv.axon_hooks" in sys.modules:
        return
    try:
        from trn_agent_boot.trn_boot import _ntff_profile_via_ctypes

        hook = _ntff_profile_via_ctypes("/opt/axon/libaxon_pjrt.so")
    except Exception:
        hook = None
    mod = types.ModuleType("antenv.axon_hooks")
    mod.get_axon_ntff_profile_hook = lambda: hook
    mod.set_axon_ntff_profile_hook = lambda h: None
    sys.modules["antenv.axon_hooks"] = mod


_install_ntff_hook()

X_BUFS = 3
PIPE_DEPTH = 1


def build_nc():
    nc = bacc.Bacc(None, target_bir_lowering=False, debug=False)

    xt = nc.declare_dram_parameter("xt", [128, (BT // TC) * KC * TC], F16, isOutput=False)
    wqkv = nc.declare_dram_parameter("wqkv", [128, KC * 384], F16, isOutput=False)
    wout = nc.declare_dram_parameter("wout", [128, C], F16, isOutput=False)
    cs = nc.declare_dram_parameter("cs", [128, 2 * T], F32, isOutput=False)
    ones = nc.declare_dram_parameter("ones", [128, 64], F16, isOutput=False)
    outT = nc.declare_dram_parameter("outT", [C, BT], F16, isOutput=True)

    with tile.TileContext(nc) as tc:
        with (
            tc.sbuf_pool(name="statics", bufs=1) as statics,
            tc.sbuf_pool(name="pool_x", bufs=X_BUFS) as pool_x,
            tc.sbuf_pool(name="pool_rope", bufs=2) as pool_rope,
            tc.sbuf_pool(name="pool_qk", bufs=2) as pool_qk,
            tc.sbuf_pool(name="pool_v", bufs=2) as pool_v,
            tc.sbuf_pool(name="pool_y", bufs=2) as pool_y,
            tc.sbuf_pool(name="pool_vs", bufs=2) as pool_vs,
            tc.sbuf_pool(name="pool_p", bufs=6) as pool_p,
            tc.sbuf_pool(name="pool_o", bufs=4) as pool_o,
            tc.sbuf_pool(name="pool_rb", bufs=2) as pool_rb,
            tc.psum_pool(name="ps_mm", bufs=2) as ps_mm,
            tc.psum_pool(name="ps_st", bufs=2) as ps_st,
            tc.psum_pool(name="ps_y", bufs=2) as ps_y,
        ):
            ident = statics.tile([128, 128], F16)
            make_identity(nc, ident)
            # mask_t[ch, col] = MASK_VAL where col < ch else 0 (strict tril)
            mask_t = statics.tile([128, 128], F16)
            nc.gpsimd.memset(mask_t, 0.0)
            nc.gpsimd.affine_select(
                out=mask_t, in_=mask_t, pattern=[[1, 128]],
                channel_multiplier=-1, base=0,
                compare_op=mybir.AluOpType.is_ge, fill=MASK_VAL,
            )

            wqkv_sb = statics.tile([128, KC * 384], F16)
            nc.sync.dma_start(out=wqkv_sb, in_=wqkv[:, :])
            # deferred statics: emitted after the first xt chunk's DMA so the
            # first QKV matmuls aren't queued behind the table loads
            wout_sb = statics.tile([128, C], F16)
            cos_sb = statics.tile([128, T], F32)
            sin_sb = statics.tile([128, T], F32)
            statics_emitted = []

            def emit_deferred_statics():
                if statics_emitted:
                    return
                statics_emitted.append(1)
                nc.sync.dma_start(out=cos_sb, in_=cs[:, 0:T])
                nc.sync.dma_start(out=sin_sb, in_=cs[:, T : 2 * T])
                nc.sync.dma_start(out=wout_sb, in_=wout[:, :])

            state = {}

            def qkv_pieces(b, tci):
                """Generator: emits the QKV+RoPE work for chunk (b, tci) in
                small pieces so it can be interleaved into the previous
                attention chunk's step loop (keeps the PE fed while ACT runs
                exp)."""
                g = 4 * b + tci
                if tci == 0:
                    state[b] = (
                        pool_qk.tile([128, T], F16, tag="q", name=f"q_{b}"),
                        pool_qk.tile([128, T], F16, tag="k", name=f"k_{b}"),
                        pool_v.tile([128, 256 * NKJ], F16, tag="v", name=f"v_{b}"),
                        pool_y.tile([128, T], F16, tag="yt", name=f"y_{b}"),
                    )
                q_t, k_t, v_all, y_t = state[b]
                xt_sb = pool_x.tile([128, KC, TC], F16, tag="x", name=f"xt_{g}")
                nc.sync.dma_start(
                    out=xt_sb,
                    in_=xt[:, g * KC * TC : (g + 1) * KC * TC].rearrange(
                        "p (kc n) -> p kc n", n=TC
                    ),
                )
                emit_deferred_statics()
                if tci == 0:
                    # ones columns of v_all (64 per head per 256-block)
                    vm = v_all.rearrange("p (m c) -> p m c", c=128)
                    ones_bc = bass.AP(
                        tensor=ones, offset=0, ap=[[64, 128], [0, 2 * NKJ], [1, 64]]
                    )
                    nc.sync.dma_start(out=vm[:, :, 64:128], in_=ones_bc)
                yield
                # V first: its PSUM slot is freed by the copy below while
                # E/O are still being produced (lets ps_mm run with bufs=2)
                psv = ps_mm.tile([128, TC], F32, tag="mm", name=f"psv_{g}")
                pse = ps_mm.tile([128, TC], F32, tag="mm", name=f"pse_{g}")
                pso = ps_mm.tile([128, TC], F32, tag="mm", name=f"pso_{g}")
                for mi, ps in ((2, psv), (0, pse), (1, pso)):
                    for kc in range(KC):
                        nc.tensor.matmul(
                            ps,
                            wqkv_sb[
                                :, kc * 384 + 128 * mi : kc * 384 + 128 * (mi + 1)
                            ],
                            xt_sb[:, kc, :],
                            start=(kc == 0),
                            stop=(kc == KC - 1),
                        )
                        yield
                # v: PSUM -> SBUF fp16, then transpose 128x128 blocks into v_all
                v_sb = pool_vs.tile([128, TC], F16, tag="vs", name=f"vsb_{g}")
                nc.scalar.activation(
                    out=v_sb, in_=psv, func=mybir.ActivationFunctionType.Copy
                )
                va = v_all.rearrange("p (n h c) -> p n h c", h=2, c=128)
                for s in range(4):
                    j = 4 * tci + s
                    tr = ps_mm.tile([128, 128], F16, tag="mm", name=f"tr_{g}_{s}")
                    nc.tensor.transpose(
                        tr, v_sb[:, 128 * s : 128 * (s + 1)], ident
                    )
                    nc.scalar.activation(
                        out=va[:, j, :, 0:64],
                        in_=tr.rearrange("p (h c) -> p h c", h=2),
                        func=mybir.ActivationFunctionType.Copy,
                    )
                    yield

                # RoPE (all DVE; fp16 outputs)
                c_sl = cos_sb[:, tci * TC : (tci + 1) * TC]
                s_sl = sin_sb[:, tci * TC : (tci + 1) * TC]
                tEC = pool_rope.tile([128, TC], F16, tag="tEC", name=f"tEC_{g}")
                nc.vector.tensor_mul(out=tEC, in0=pse, in1=c_sl)
                tES = pool_rope.tile([128, TC], F16, tag="tES", name=f"tES_{g}")
                nc.vector.tensor_mul(out=tES, in0=pse, in1=s_sl)
                yield
                tOS = pool_rope.tile([128, TC], F16, tag="tOS", name=f"tOS_{g}")
                nc.vector.tensor_mul(out=tOS, in0=pso, in1=s_sl)
                tOC = pool_rope.tile([128, TC], F16, tag="tOC", name=f"tOC_{g}")
                nc.vector.tensor_mul(out=tOC, in0=pso, in1=c_sl)
                yield

                sl = slice(tci * TC, (tci + 1) * TC)
                # q_t rows [h0e|h0o|h1e|h1o]; E rows [q0e|q1e|k0e|k1e]
                # q add/subs first: attention chunk tci only needs q ready
                for h in range(2):
                    he = slice(32 * h, 32 * (h + 1))
                    nc.vector.tensor_sub(
                        out=q_t[64 * h : 64 * h + 32, sl],
                        in0=tEC[he], in1=tOS[he],
                    )
                    nc.vector.tensor_add(
                        out=q_t[64 * h + 32 : 64 * h + 64, sl],
                        in0=tES[he], in1=tOC[he],
                    )
                yield
                for h in range(2):
                    ke_ = slice(64 + 32 * h, 64 + 32 * (h + 1))
                    nc.vector.tensor_sub(
                        out=k_t[64 * h : 64 * h + 32, sl],
                        in0=tEC[ke_], in1=tOS[ke_],
                    )
                    nc.vector.tensor_add(
                        out=k_t[64 * h + 32 : 64 * h + 64, sl],
                        in0=tES[ke_], in1=tOC[ke_],
                    )
                yield

            _DONE = object()

            def pull(gen, n):
                if gen is None:
                    return
                for _ in range(n):
                    if next(gen, _DONE) is _DONE:
                        return

            def drain(gen):
                if gen is not None:
                    for _ in gen:
                        pass

            def attention_chunk(b, i, feed):
                """Scores/softmax/PV for query chunk i of batch b, pulling
                pieces of the NEXT chunk's QKV work between steps."""
                q_t, k_t, v_all, y_t = state[b]
                nj = 4 * i + 4
                yaccs = {}
                for h in range(2):
                    yaccs[h] = ps_y.tile(
                        [128, TC], F32, tag="y", name=f"yacc_{b}_{i}_{h}"
                    )

                def st_of(j):
                    r = j - 4 * i
                    return 128 * r if r > 0 else 0

                n_steps = nj + PIPE_DEPTH
                per_step = (31 + n_steps - 1) // n_steps

                # software-pipelined by one step: PE issues S(j),S(j),
                # PV(j-1),PV(j-1) back-to-back while exp(j) runs on ACT
                p_tiles = {}
                for j in range(n_steps):
                    if j < nj:
                        st = st_of(j)
                        r = j - 4 * i
                        ksl = slice(128 * j, 128 * (j + 1))
                        qsl = slice(TC * i + st, TC * (i + 1))
                        ps_s = ps_st.tile(
                            [128, 2, TC], F32, tag="st", name=f"s_{b}_{i}_{j}"
                        )
                        for h in range(2):
                            hs = slice(64 * h, 64 * (h + 1))
                            nc.tensor.matmul(
                                ps_s[:, h, st:], k_t[hs, ksl], q_t[hs, qsl],
                                start=True, stop=(r < 0),
                            )
                            if r >= 0:
                                # accumulate -240*tril into the diagonal
                                # 128x128 block (causal mask, pre-exp)
                                nc.tensor.matmul(
                                    ps_s[:, h, st : st + 128], ident, mask_t,
                                    start=False, stop=True,
                                )
                        p_sb = pool_p.tile(
                            [128, 2, TC], F16, tag="p", name=f"p_{b}_{i}_{j}"
                        )
                        p_tiles[j] = p_sb
                        # one exp for both heads (2-bank PSUM source)
                        nc.scalar.activation(
                            out=p_sb[:, :, st:], in_=ps_s[:, :, st:],
                            func=mybir.ActivationFunctionType.Exp,
                            scale=0.125,
                        )
                    if j >= PIPE_DEPTH:
                        jp = j - PIPE_DEPTH
                        st = st_of(jp)
                        pp = p_tiles.pop(jp)
                        for h in range(2):
                            nc.tensor.matmul(
                                yaccs[h][:, st:],
                                v_all[
                                    :, 256 * jp + 128 * h : 256 * jp + 128 * (h + 1)
                                ],
                                pp[:, h, st:],
                                start=(jp == 0),
                                stop=(jp == nj - 1),
                            )
                    pull(feed, per_step)
                for h in range(2):
                    rb = pool_rb.tile(
                        [128, TC], F32, tag="rb", name=f"rb_{b}_{i}_{h}"
                    )
                    # full-tile: the custom op mislowers nonzero base
                    # partitions; rows 0:64 are unused garbage recips
                    nc.vector.reciprocal_approx_fast(out=rb, in_=yaccs[h])
                    nc.vector.tensor_mul(
                        out=y_t[64 * h : 64 * (h + 1), TC * i : TC * (i + 1)],
                        in0=yaccs[h][0:64, :],
                        in1=rb[64:128],
                    )
                drain(feed)
                outproj_chunk(b, i, y_t)

            def outproj_chunk(b, tci, y_t):
                g = 4 * b + tci
                for cc in range(KC):
                    ps = ps_mm.tile([128, TC], F32, tag="mm", name=f"op_{g}_{cc}")
                    nc.tensor.matmul(
                        ps,
                        wout_sb[:, 128 * cc : 128 * (cc + 1)],
                        y_t[:, TC * tci : TC * (tci + 1)],
                        start=True,
                        stop=True,
                    )
                    o_sb = pool_o.tile([128, TC], F16, tag="o", name=f"o_{g}_{cc}")
                    if cc % 2 == 0:
                        nc.scalar.activation(
                            out=o_sb, in_=ps,
                            func=mybir.ActivationFunctionType.Copy,
                        )
                    else:
                        nc.vector.tensor_copy(out=o_sb, in_=ps)
                    nc.sync.dma_start(
                        out=outT[
                            128 * cc : 128 * (cc + 1), g * TC : (g + 1) * TC
                        ],
                        in_=o_sb,
                    )

            chunks = [(b, i) for b in range(B) for i in range(NQI)]
            gen = qkv_pieces(*chunks[0])
            drain(gen)
            for ci, (b, i) in enumerate(chunks):
                feed = qkv_pieces(*chunks[ci + 1]) if ci + 1 < len(chunks) else None
                attention_chunk(b, i, feed)

    nc.compile()
    return nc


_NC_CACHE = None


def _get_nc():
    global _NC_CACHE
    if _NC_CACHE is None:
        _NC_CACHE = build_nc()
    return _NC_CACHE


def _host_prep(x, qkv_w, out_w):
    x = np.asarray(x, dtype=np.float32)
    qkv_w = np.asarray(qkv_w, dtype=np.float32)
    out_w = np.asarray(out_w, dtype=np.float32)

    # xt[p, ((g*KC)+kc)*TC + n] = x[g*TC + n, kc*128 + p] - one contiguous
    # line per (partition, chunk) for the per-chunk DMA
    xt = np.ascontiguousarray(
        x.reshape(BT // TC, TC, KC, 128).transpose(3, 0, 2, 1).reshape(128, -1)
    ).astype(np.float16)

    # rope tables: row p uses frequency index p % 32
    t_idx = np.arange(T, dtype=np.float64)
    inv_freq = 1.0 / (10000.0 ** (np.arange(0, D, 2, dtype=np.float64) / D))  # 32
    ang = np.outer(np.tile(inv_freq, 4), t_idx)  # [128, T]
    cs = np.concatenate(
        [np.cos(ang), np.sin(ang)], axis=1
    ).astype(np.float32)  # [128, 2T]

    ones = np.ones((128, 64), np.float16)

    in_maps = []
    for core in range(N_CORES):
        h0 = 2 * core
        h1 = h0 + 1
        ev = np.arange(0, D, 2)
        od = np.arange(1, D, 2)
        e_rows = np.concatenate(
            [h0 * D + ev, h1 * D + ev, C + h0 * D + ev, C + h1 * D + ev]
        )
        o_rows = np.concatenate(
            [h0 * D + od, h1 * D + od, C + h0 * D + od, C + h1 * D + od]
        )
        v_rows = np.concatenate(
            [2 * C + h0 * D + np.arange(D), 2 * C + h1 * D + np.arange(D)]
        )
        rows = np.concatenate([e_rows, o_rows, v_rows])  # [384]
        w_part = qkv_w[rows]  # [384, C]
        # wqkv[p, kc*384 + m] = w_part[m, kc*128 + p]
        wqkv_c = np.ascontiguousarray(
            w_part.T.reshape(KC, 128, 384).transpose(1, 0, 2).reshape(128, KC * 384)
        ).astype(np.float16)
        cols = np.concatenate([h0 * D + np.arange(D), h1 * D + np.arange(D)])
        wout_c = np.ascontiguousarray(out_w[:, cols].T).astype(np.float16)  # [128, C]
        in_maps.append(
            {"xt": xt, "wqkv": wqkv_c, "wout": wout_c, "cs": cs, "ones": ones}
        )
    return in_maps


def _run(in_maps, trace=False):
    nc = _get_nc()
    return run_bass_kernel_spmd(
        nc, in_maps, core_ids=list(range(N_CORES)), trace=trace
    )


def kernel(x, qkv_w, out_w, _trace=False, _results_box=None):
    in_maps = _host_prep(x, qkv_w, out_w)
    res = _run(in_maps, trace=_trace)
    if _results_box is not None:
        _results_box.append(res)
    acc = np.zeros((C, BT), np.float32)
    for r in res.results:
        acc += r["outT"].astype(np.float32)
    out = acc.T.reshape(B, T, C)
    return np.ascontiguousarray(out)


# revision 7
# speedup vs baseline: 1.3068x; 1.0339x over previous
"""Causal self-attention with RoPE (B=2, T=2048, C=1024, H=16, D=64) on 8
Trainium2 NeuronCores.

Sharding: tensor-parallel over heads - each core owns 2 heads (QKV and output
projections sliced on the head axis); the per-core partial outputs (full
[C, B*T] each, fp16) are summed on the host in fp32.

v3 changes vs the fp32r baseline:
  - fp16 datapath end to end (halves DMA, LDWEIGHTS, removes fp32r small-N
    penalty, enables DVE 2x 16-bit mode for the RoPE add/subs).
  - causal masking is a PE matmul accumulate (identity @ (-240*tril) into the
    diagonal score blocks before exp) instead of a gpsimd affine_select after
    exp - the softmax chain is now S -> exp -> PV with no gpsimd hop.
  - QKV chunk i+1 is emitted piecewise INTO attention chunk i's step loop so
    the tensor engine always has independent matmuls to chew on while ACT
    works through exp (keeps the PE p-state high); the exp for both heads is
    one ACT instruction on a 2-bank PSUM tile [128, 2, 512].
  - out-proj PSUM->SBUF fp16 copies split between gpsimd and DVE; ACT keeps
    only exp + v copies.

Per-core layout (everything transposed: features on partitions, tokens free):
  xT [1024, 4096] fp16 -> QKV chunks via PE matmul, W rows pre-permuted into
  E/O/V 128-row groups; RoPE on DVE; scores S^T per (kj, head); PV
  accumulates y^T plus denominator rows via 64 ones columns in v; out proj
  partial [1024, 4096] fp16 DMA'd out.
"""

import sys
import types

import numpy as np

import concourse.bass as bass
import concourse.tile as tile
from concourse import bacc
from concourse import mybir
from concourse.bass_utils import run_bass_kernel_spmd
from concourse.masks import make_identity

F32 = mybir.dt.float32
F16 = mybir.dt.float16

B = 2
T = 2048
C = 1024
D = 64
N_CORES = 8
BT = B * T              # 4096
TC = 512                # token chunk (free dim of most matmuls)
NQI = T // TC           # 4 qi chunks per batch
NKJ = T // 128          # 16 kj chunks per batch
KC = C // 128           # 8 contraction chunks for the projections
MASK_VAL = -240.0       # exp(0.125*(s+MASK_VAL)) underflows fp16 to 0


def _install_ntff_hook():
    """bass_utils imports antenv.axon_hooks when tracing; this image lacks it.
    Recreate it from the ctypes NTFF driver so trace=True works."""
    if "antenv.axon_hooks" in sys.modules:
        return
    try:
        from trn_agent_boot.trn_boot import _ntff_profile_via_ctypes

        hook = _ntff_profile_via_ctypes("/opt/axon/libaxon_pjrt.so")
    except Exception:
        hook = None
    mod = types.ModuleType("antenv.axon_hooks")
    mod.get_axon_ntff_profile_hook = lambda: hook
    mod.set_axon_ntff_profile_hook = lambda h: None
    sys.modules["antenv.axon_hooks"] = mod


_install_ntff_hook()

X_BUFS = 3
PIPE_DEPTH = 1


def build_nc():
    nc = bacc.Bacc(None, target_bir_lowering=False, debug=False)

    xt = nc.declare_dram_parameter("xt", [128, (BT // TC) * KC * TC], F16, isOutput=False)
    wqkv = nc.declare_dram_parameter("wqkv", [128, KC * 384], F16, isOutput=False)
    wout = nc.declare_dram_parameter("wout", [128, C], F16, isOutput=False)
    cs = nc.declare_dram_parameter("cs", [128, 2 * T], F32, isOutput=False)
    ones = nc.declare_dram_parameter("ones", [128, 64], F16, isOutput=False)
    outT = nc.declare_dram_parameter("outT", [C, BT], F16, isOutput=True)

    with tile.TileContext(nc) as tc:
        with (
            tc.sbuf_pool(name="statics", bufs=1) as statics,
            tc.sbuf_pool(name="pool_x", bufs=X_BUFS) as pool_x,
            tc.sbuf_pool(name="pool_rope", bufs=2) as pool_rope,
            tc.sbuf_pool(name="pool_qk", bufs=2) as pool_qk,
            tc.sbuf_pool(name="pool_v", bufs=2) as pool_v,
            tc.sbuf_pool(name="pool_y", bufs=2) as pool_y,
            tc.sbuf_pool(name="pool_vs", bufs=2) as pool_vs,
            tc.sbuf_pool(name="pool_p", bufs=6) as pool_p,
            tc.sbuf_pool(name="pool_o", bufs=4) as pool_o,
            tc.sbuf_pool(name="pool_rb", bufs=2) as pool_rb,
            tc.psum_pool(name="ps_mm", bufs=2) as ps_mm,
            tc.psum_pool(name="ps_st", bufs=2) as ps_st,
            tc.psum_pool(name="ps_y", bufs=2) as ps_y,
        ):
            ident = statics.tile([128, 128], F16)
            make_identity(nc, ident)
            # mask_t[ch, col] = MASK_VAL where col < ch else 0 (strict tril)
            mask_t = statics.tile([128, 128], F16)
            nc.gpsimd.memset(mask_t, 0.0)
            nc.gpsimd.affine_select(
                out=mask_t, in_=mask_t, pattern=[[1, 128]],
                channel_multiplier=-1, base=0,
                compare_op=mybir.AluOpType.is_ge, fill=MASK_VAL,
            )

            wqkv_sb = statics.tile([128, KC * 384], F16)
            nc.sync.dma_start(out=wqkv_sb, in_=wqkv[:, :])
            # deferred statics: emitted after the first xt chunk's DMA so the
            # first QKV matmuls aren't queued behind the table loads
            wout_sb = statics.tile([128, C], F16)
            cos_sb = statics.tile([128, T], F32)
            sin_sb = statics.tile([128, T], F32)
            statics_emitted = []

            def emit_deferred_statics():
                if statics_emitted:
                    return
                statics_emitted.append(1)
                nc.sync.dma_start(out=cos_sb, in_=cs[:, 0:T])
                nc.sync.dma_start(out=sin_sb, in_=cs[:, T : 2 * T])
                nc.sync.dma_start(out=wout_sb, in_=wout[:, :])

            state = {}

            def qkv_pieces(b, tci):
                """Generator: emits the QKV+RoPE work for chunk (b, tci) in
                small pieces so it can be interleaved into the previous
                attention chunk's step loop (keeps the PE fed while ACT runs
                exp)."""
                g = 4 * b + tci
                if tci == 0:
                    state[b] = (
                        pool_qk.tile([128, T], F16, tag="q", name=f"q_{b}"),
                        pool_qk.tile([128, T], F16, tag="k", name=f"k_{b}"),
                        pool_v.tile([128, 256 * NKJ], F16, tag="v", name=f"v_{b}"),
                        pool_y.tile([128, T], F16, tag="yt", name=f"y_{b}"),
                    )
                q_t, k_t, v_all, y_t = state[b]
                xt_sb = pool_x.tile([128, KC, TC], F16, tag="x", name=f"xt_{g}")
                nc.sync.dma_start(
                    out=xt_sb,
                    in_=xt[:, g * KC * TC : (g + 1) * KC * TC].rearrange(
                        "p (kc n) -> p kc n", n=TC
                    ),
                )
                emit_deferred_statics()
                if tci == 0:
                    # ones columns of v_all (64 per head per 256-block)
                    vm = v_all.rearrange("p (m c) -> p m c", c=128)
                    ones_bc = bass.AP(
                        tensor=ones, offset=0, ap=[[64, 128], [0, 2 * NKJ], [1, 64]]
                    )
                    nc.sync.dma_start(out=vm[:, :, 64:128], in_=ones_bc)
                yield
                # V first: its PSUM slot is freed by the copy below while
                # E/O are still being produced (lets ps_mm run with bufs=2)
                psv = ps_mm.tile([128, TC], F32, tag="mm", name=f"psv_{g}")
                pse = ps_mm.tile([128, TC], F32, tag="mm", name=f"pse_{g}")
                pso = ps_mm.tile([128, TC], F32, tag="mm", name=f"pso_{g}")
                for mi, ps in ((2, psv), (0, pse), (1, pso)):
                    for kc in range(KC):
                        nc.tensor.matmul(
                            ps,
                            wqkv_sb[
                                :, kc * 384 + 128 * mi : kc * 384 + 128 * (mi + 1)
                            ],
                            xt_sb[:, kc, :],
                            start=(kc == 0),
                            stop=(kc == KC - 1),
                        )
                        yield
                # v: PSUM -> SBUF fp16, then transpose 128x128 blocks into v_all
                v_sb = pool_vs.tile([128, TC], F16, tag="vs", name=f"vsb_{g}")
                nc.scalar.activation(
                    out=v_sb, in_=psv, func=mybir.ActivationFunctionType.Copy
                )
                va = v_all.rearrange("p (n h c) -> p n h c", h=2, c=128)
                for s in range(4):
                    j = 4 * tci + s
                    tr = ps_mm.tile([128, 128], F16, tag="mm", name=f"tr_{g}_{s}")
                    nc.tensor.transpose(
                        tr, v_sb[:, 128 * s : 128 * (s + 1)], ident
                    )
                    nc.vector.tensor_copy(
                        out=va[:, j, :, 0:64],
                        in_=tr.rearrange("p (h c) -> p h c", h=2),
                    )
                    yield

                # RoPE (all DVE; fp16 outputs)
                c_sl = cos_sb[:, tci * TC : (tci + 1) * TC]
                s_sl = sin_sb[:, tci * TC : (tci + 1) * TC]
                tEC = pool_rope.tile([128, TC], F16, tag="tEC", name=f"tEC_{g}")
                nc.vector.tensor_mul(out=tEC, in0=pse, in1=c_sl)
                tES = pool_rope.tile([128, TC], F16, tag="tES", name=f"tES_{g}")
                nc.vector.tensor_mul(out=tES, in0=pse, in1=s_sl)
                yield
                tOS = pool_rope.tile([128, TC], F16, tag="tOS", name=f"tOS_{g}")
                nc.vector.tensor_mul(out=tOS, in0=pso, in1=s_sl)
                tOC = pool_rope.tile([128, TC], F16, tag="tOC", name=f"tOC_{g}")
                nc.vector.tensor_mul(out=tOC, in0=pso, in1=c_sl)
                yield

                sl = slice(tci * TC, (tci + 1) * TC)
                # q_t rows [h0e|h0o|h1e|h1o]; E rows [q0e|q1e|k0e|k1e]
                # q add/subs first: attention chunk tci only needs q ready
                for h in range(2):
                    he = slice(32 * h, 32 * (h + 1))
                    nc.vector.tensor_sub(
                        out=q_t[64 * h : 64 * h + 32, sl],
                        in0=tEC[he], in1=tOS[he],
                    )
                    nc.vector.tensor_add(
                        out=q_t[64 * h + 32 : 64 * h + 64, sl],
                        in0=tES[he], in1=tOC[he],
                    )
                yield
                for h in range(2):
                    ke_ = slice(64 + 32 * h, 64 + 32 * (h + 1))
                    nc.vector.tensor_sub(
                        out=k_t[64 * h : 64 * h + 32, sl],
                        in0=tEC[ke_], in1=tOS[ke_],
                    )
                    nc.vector.tensor_add(
                        out=k_t[64 * h + 32 : 64 * h + 64, sl],
                        in0=tES[ke_], in1=tOC[ke_],
                    )
                yield

            _DONE = object()

            def pull(gen, n):
                if gen is None:
                    return
                for _ in range(n):
                    if next(gen, _DONE) is _DONE:
                        return

            def drain(gen):
                if gen is not None:
                    for _ in gen:
                        pass

            def attention_chunk(b, i, feed):
                """Scores/softmax/PV for query chunk i of batch b, pulling
                pieces of the NEXT chunk's QKV work between steps."""
                q_t, k_t, v_all, y_t = state[b]
                nj = 4 * i + 4
                yaccs = {}
                for h in range(2):
                    yaccs[h] = ps_y.tile(
                        [128, TC], F32, tag="y", name=f"yacc_{b}_{i}_{h}"
                    )

                def st_of(j):
                    r = j - 4 * i
                    return 128 * r if r > 0 else 0

                n_steps = nj + PIPE_DEPTH
                per_step = (34 + n_steps - 1) // n_steps

                # software-pipelined by one step: PE issues S(j),S(j),
                # PV(j-1),PV(j-1) back-to-back while exp(j) runs on ACT
                p_tiles = {}
                for j in range(n_steps):
                    if j < nj:
                        st = st_of(j)
                        r = j - 4 * i
                        ksl = slice(128 * j, 128 * (j + 1))
                        qsl = slice(TC * i + st, TC * (i + 1))
                        ps_s = ps_st.tile(
                            [128, 2, TC], F32, tag="st", name=f"s_{b}_{i}_{j}"
                        )
                        for h in range(2):
                            hs = slice(64 * h, 64 * (h + 1))
                            nc.tensor.matmul(
                                ps_s[:, h, st:], k_t[hs, ksl], q_t[hs, qsl],
                                start=True, stop=(r < 0),
                            )
                            if r >= 0:
                                # accumulate -240*tril into the diagonal
                                # 128x128 block (causal mask, pre-exp)
                                nc.tensor.matmul(
                                    ps_s[:, h, st : st + 128], ident, mask_t,
                                    start=False, stop=True,
                                )
                        p_sb = pool_p.tile(
                            [128, 2, TC], F16, tag="p", name=f"p_{b}_{i}_{j}"
                        )
                        p_tiles[j] = p_sb
                        # one exp for both heads (2-bank PSUM source)
                        nc.scalar.activation(
                            out=p_sb[:, :, st:], in_=ps_s[:, :, st:],
                            func=mybir.ActivationFunctionType.Exp,
                            scale=0.125,
                        )
                    if j >= PIPE_DEPTH:
                        jp = j - PIPE_DEPTH
                        st = st_of(jp)
                        pp = p_tiles.pop(jp)
                        for h in range(2):
                            nc.tensor.matmul(
                                yaccs[h][:, st:],
                                v_all[
                                    :, 256 * jp + 128 * h : 256 * jp + 128 * (h + 1)
                                ],
                                pp[:, h, st:],
                                start=(jp == 0),
                                stop=(jp == nj - 1),
                            )
                    pull(feed, per_step)
                for h in range(2):
                    rb = pool_rb.tile(
                        [128, TC], F32, tag="rb", name=f"rb_{b}_{i}_{h}"
                    )
                    # full-tile: the custom op mislowers nonzero base
                    # partitions; rows 0:64 are unused garbage recips
                    nc.vector.reciprocal_approx_fast(out=rb, in_=yaccs[h])
                    nc.vector.tensor_mul(
                        out=y_t[64 * h : 64 * (h + 1), TC * i : TC * (i + 1)],
                        in0=yaccs[h][0:64, :],
                        in1=rb[64:128],
                    )
                drain(feed)
                outproj_chunk(b, i, y_t)

            def outproj_chunk(b, tci, y_t):
                g = 4 * b + tci
                for cc in range(KC):
                    ps = ps_mm.tile([128, TC], F32, tag="mm", name=f"op_{g}_{cc}")
                    nc.tensor.matmul(
                        ps,
                        wout_sb[:, 128 * cc : 128 * (cc + 1)],
                        y_t[:, TC * tci : TC * (tci + 1)],
                        start=True,
                        stop=True,
                    )
                    o_sb = pool_o.tile([128, TC], F16, tag="o", name=f"o_{g}_{cc}")
                    if cc % 2 == 0:
                        nc.scalar.activation(
                            out=o_sb, in_=ps,
                            func=mybir.ActivationFunctionType.Copy,
                        )
                    else:
                        nc.vector.tensor_copy(out=o_sb, in_=ps)
                    nc.sync.dma_start(
                        out=outT[
                            128 * cc : 128 * (cc + 1), g * TC : (g + 1) * TC
                        ],
                        in_=o_sb,
                    )

            chunks = [(b, i) for b in range(B) for i in range(NQI)]
            gen = qkv_pieces(*chunks[0])
            drain(gen)
            for ci, (b, i) in enumerate(chunks):
                feed = qkv_pieces(*chunks[ci + 1]) if ci + 1 < len(chunks) else None
                attention_chunk(b, i, feed)

    nc.compile()
    return nc


_NC_CACHE = None


def _get_nc():
    global _NC_CACHE
    if _NC_CACHE is None:
        _NC_CACHE = build_nc()
    return _NC_CACHE


def _host_prep(x, qkv_w, out_w):
    x = np.asarray(x, dtype=np.float32)
    qkv_w = np.asarray(qkv_w, dtype=np.float32)
    out_w = np.asarray(out_w, dtype=np.float32)

    # xt[p, ((g*KC)+kc)*TC + n] = x[g*TC + n, kc*128 + p] - one contiguous
    # line per (partition, chunk) for the per-chunk DMA
    xt = np.ascontiguousarray(
        x.reshape(BT // TC, TC, KC, 128).transpose(3, 0, 2, 1).reshape(128, -1)
    ).astype(np.float16)

    # rope tables: row p uses frequency index p % 32
    t_idx = np.arange(T, dtype=np.float64)
    inv_freq = 1.0 / (10000.0 ** (np.arange(0, D, 2, dtype=np.float64) / D))  # 32
    ang = np.outer(np.tile(inv_freq, 4), t_idx)  # [128, T]
    cs = np.concatenate(
        [np.cos(ang), np.sin(ang)], axis=1
    ).astype(np.float32)  # [128, 2T]

    ones = np.ones((128, 64), np.float16)

    in_maps = []
    for core in range(N_CORES):
        h0 = 2 * core
        h1 = h0 + 1
        ev = np.arange(0, D, 2)
        od = np.arange(1, D, 2)
        e_rows = np.concatenate(
            [h0 * D + ev, h1 * D + ev, C + h0 * D + ev, C + h1 * D + ev]
        )
        o_rows = np.concatenate(
            [h0 * D + od, h1 * D + od, C + h0 * D + od, C + h1 * D + od]
        )
        v_rows = np.concatenate(
            [2 * C + h0 * D + np.arange(D), 2 * C + h1 * D + np.arange(D)]
        )
        rows = np.concatenate([e_rows, o_rows, v_rows])  # [384]
        w_part = qkv_w[rows]  # [384, C]
        # wqkv[p, kc*384 + m] = w_part[m, kc*128 + p]
        wqkv_c = np.ascontiguousarray(
            w_part.T.reshape(KC, 128, 384).transpose(1, 0, 2).reshape(128, KC * 384)
        ).astype(np.float16)
        cols = np.concatenate([h0 * D + np.arange(D), h1 * D + np.arange(D)])
        wout_c = np.ascontiguousarray(out_w[:, cols].T).astype(np.float16)  # [128, C]
        in_maps.append(
            {"xt": xt, "wqkv": wqkv_c, "wout": wout_c, "cs": cs, "ones": ones}
        )
    return in_maps


def _run(in_maps, trace=False):
    nc = _get_nc()
    return run_bass_kernel_spmd(
        nc, in_maps, core_ids=list(range(N_CORES)), trace=trace
    )


def kernel(x, qkv_w, out_w, _trace=False, _results_box=None):
    in_maps = _host_prep(x, qkv_w, out_w)
    res = _run(in_maps, trace=_trace)
    if _results_box is not None:
        _results_box.append(res)
    acc = np.zeros((C, BT), np.float32)
    for r in res.results:
        acc += r["outT"].astype(np.float32)
    out = acc.T.reshape(B, T, C)
    return np.ascontiguousarray(out)
